# revision 2
# baseline (speedup 1.0000x reference)
"""Trainium2 Bass kernel v2 for nn_BERT_LSTM_CRF — optimized P2 recurrence.

Changes vs v1 baseline (3.06ms):
 - W1 fused into Wih on host: F = W1' @ Wih^T [768, 2048] (softmax(arch)
   folded into W1' rows); P1 = gather -> transpose -> one fused matmul.
 - Whh stored fp8e4 (x64 scale) -> FWL weight loads at 4 elem/cycle; the
   1/64 descale is folded into the sigmoid/tanh activation input scale
   (xg and gate bias are computed pre-scaled by 64 via F and d).
 - xg preloaded into PSUM one step ahead; gate matmuls accumulate on top
   (start=False) -> no per-step DVE gate add.
 - Gate tiles reordered [i i f f o o g g | per hid-half] so one ACT does
   a half's sigmoids; ACT reads PSUM directly.
 - Pointwise split in two hid-halves; next step's matmuls are emitted
   kt-pair-ordered so they start as soon as the matching h half exists.
 - P1 work for chunk c+1 interleaved into P2's per-step PE idle gaps.

Topology unchanged: cores 0-3 fwd / 4-7 rev (host-flipped), batch 8/core.
"""

import contextlib
import ctypes
import os
import sys
import types

sys.path.insert(0, "/opt/trn_rl_repo")

import numpy as np
import ml_dtypes

import concourse.bacc as bacc
import concourse.bass as bass
import concourse.mybir as mybir
import concourse.tile as tile
from concourse.bass_utils import run_bass_kernel_spmd
from concourse.masks import make_identity

F32 = mybir.dt.float32
BF16 = mybir.dt.bfloat16
FP8 = mybir.dt.float8e4
I32 = mybir.dt.int32
AF = mybir.ActivationFunctionType
ALU = mybir.AluOpType

P = 128
DE = 256          # embedding dim per table
NE = 3            # number of tables
HID = 512
G4 = 4 * HID      # 2048 gate dim
FEAT = NE * DE    # 768 fused input dim
TAGP2 = 22
B_LOC = 8         # batch rows per core
N_CORES = 8
CH_S = 64         # seq steps per chunk
WHH_SCALE = 64.0  # fp8 scale for Whh/xg/bias, descaled in the activations

# gate tile order within a step's PSUM columns, per hid-half:
# [i0 i1 f0 f1 o0 o1 g0 g1 | i2 i3 f2 f3 o2 o3 g2 g3]
# (T, s): T in {0:i, 1:f, 2:g, 3:o}; original col block = T*512 + s*128
PERM = [(0, 0), (0, 1), (1, 0), (1, 1), (3, 0), (3, 1), (2, 0), (2, 1),
        (0, 2), (0, 3), (1, 2), (1, 3), (3, 2), (3, 3), (2, 2), (2, 3)]

LAST_EXEC_NS = None


# --------------------------------------------------------------------------
# NTFF profiling shim (antenv.axon_hooks is missing from this image).
def _install_ntff_shim():
    if "antenv.axon_hooks" in sys.modules:
        return

    def _make_hook():
        try:
            lib = ctypes.CDLL("/opt/axon/libaxon_pjrt.so")
        except OSError:
            return None
        if not hasattr(lib, "axon_start_nrt_profile"):
            return None
        lib.axon_start_nrt_profile.argtypes = [
            ctypes.POINTER(ctypes.c_int64),
            ctypes.c_size_t,
        ]
        lib.axon_start_nrt_profile.restype = ctypes.c_int64
        lib.axon_stop_nrt_profile.argtypes = [ctypes.c_char_p]
        lib.axon_stop_nrt_profile.restype = ctypes.c_int64

        @contextlib.contextmanager
        def _hook(output_dir, device_ids):
            import jax

            jax.devices()
            if device_ids:
                ids = (ctypes.c_int64 * len(device_ids))(*device_ids)
                rc = lib.axon_start_nrt_profile(ids, len(device_ids))
            else:
                rc = lib.axon_start_nrt_profile(None, 0)
            if rc != 0:
                raise RuntimeError(f"axon_start_nrt_profile rc={rc}")
            try:
                yield
            finally:
                n = lib.axon_stop_nrt_profile(str(output_dir).encode())
                if n < 0:
                    raise RuntimeError(f"axon_stop_nrt_profile rc={n}")

        return _hook

    mod = types.ModuleType("antenv.axon_hooks")
    mod.get_axon_ntff_profile_hook = _make_hook
    sys.modules["antenv.axon_hooks"] = mod


_install_ntff_shim()


# --------------------------------------------------------------------------
def build_nc(S, V, whh_fp8=True, interleave=True, num_devices=N_CORES):
    """Per-core program. S = seq len (multiple of CH_S), V = vocab."""
    n_tok = B_LOC * S
    n_tile = n_tok // P                  # 128-token tiles
    CH_TOK = CH_S * B_LOC                # 512 tokens per chunk
    n_ch = S // CH_S
    ch_tile = CH_TOK // P                # 4 token-tiles per chunk
    n_gj = n_tile * NE
    WDT = FP8 if whh_fp8 else BF16
    inv_s = 1.0 / (WHH_SCALE if whh_fp8 else 1.0)

    nc = bacc.Bacc("TRN2", target_bir_lowering=False, debug=False,
                   num_devices=num_devices)

    tables = nc.dram_tensor("tables", [NE * V, DE], F32, kind="ExternalInput")
    gidx_in = nc.dram_tensor("gidx", [P, n_gj], I32, kind="ExternalInput")
    fT_in = nc.dram_tensor("fT", [P, 6 * G4], BF16, kind="ExternalInput")
    whh_in = nc.dram_tensor("whhT", [P, 4 * G4], WDT, kind="ExternalInput")
    wout_in = nc.dram_tensor("wout", [P, 4 * TAGP2], BF16, kind="ExternalInput")
    dg_in = nc.dram_tensor("dg", [P, 16], F32, kind="ExternalInput")
    bout_in = nc.dram_tensor("boutc", [TAGP2, 1], F32, kind="ExternalInput")
    outp = nc.dram_tensor("outp", [TAGP2, n_tok], F32, kind="ExternalOutput")

    with tile.TileContext(nc) as tc:
        ctx = contextlib.ExitStack()
        with ctx:
            wper = ctx.enter_context(tc.tile_pool(name="wper", bufs=1))
            gidx_sb = wper.tile([P, n_gj], I32)
            nc.sync.dma_start(out=gidx_sb[:], in_=gidx_in.ap())
            whh_sb = wper.tile([P, 4 * G4], WDT)
            nc.sync.dma_start(out=whh_sb[:], in_=whh_in.ap())
            fT_sb = wper.tile([P, 6 * G4], BF16)
            nc.sync.dma_start(out=fT_sb[:], in_=fT_in.ap())
            wout_sb = wper.tile([P, 4 * TAGP2], BF16)
            nc.sync.dma_start(out=wout_sb[:], in_=wout_in.ap())
            dcol = wper.tile([P, 16], F32)
            nc.sync.dma_start(out=dcol[:], in_=dg_in.ap())
            bout_sb = wper.tile([TAGP2, 1], F32)
            nc.sync.dma_start(out=bout_sb[:], in_=bout_in.ap())
            ident = wper.tile([P, P], F32)
            make_identity(nc, ident[:])
            ident_bf = wper.tile([P, P], BF16)
            nc.vector.tensor_copy(out=ident_bf[:], in_=ident[:])

            hT = wper.tile([P, 4 * n_tok], BF16)
            c_sb = wper.tile([P, 4 * B_LOC], F32)
            nc.vector.memset(c_sb[:], 0.0)

            grp = ctx.enter_context(tc.tile_pool(name="grp", bufs=4))
            xtp = ctx.enter_context(tc.tile_pool(name="xtp", bufs=2))
            xgp = ctx.enter_context(tc.tile_pool(name="xgp", bufs=2))
            pwp = ctx.enter_context(tc.tile_pool(name="pwp", bufs=4))

            pctx = contextlib.ExitStack()
            pxp = pctx.enter_context(
                tc.tile_pool(name="pxp", bufs=2, space="PSUM"))
            prpa = pctx.enter_context(
                tc.tile_pool(name="prpa", bufs=3, space="PSUM"))
            prpb = pctx.enter_context(
                tc.tile_pool(name="prpb", bufs=3, space="PSUM"))

            xg_of = {}          # chunk index -> xg SBUF tile

            # ---------------- P1 emission (one chunk = list of closures) --
            def p1_ops(ci):
                """Returns (pe_ops, dve_ops): pe_ops feed the PE/DMA queues,
                dve_ops are (min_pe_emitted, fn) evac closures that must be
                emitted after their producing matmuls."""
                ops = []
                xrows = [None] * ch_tile
                xT = [None]
                px = {}

                def mk_gather(ti, e):
                    def f():
                        if e == 0:
                            xrows[ti] = grp.tile([P, FEAT], BF16, tag="xrow", name="xrow")
                        j = (ci * ch_tile + ti) * NE + e
                        nc.gpsimd.indirect_dma_start(
                            out=xrows[ti][:, e * DE:(e + 1) * DE],
                            out_offset=None,
                            in_=tables.ap(),
                            in_offset=bass.IndirectOffsetOnAxis(
                                ap=gidx_sb[:, j:j + 1], axis=0))
                    return f

                def mk_tr(ti, fc):
                    def f():
                        if ti == 0 and fc == 0:
                            xT[0] = xtp.tile([P, 6 * CH_TOK], BF16, tag="xT", name="xT")
                        nc.sync.dma_start(
                            out=xT[0][:, fc * CH_TOK + ti * P:
                                      fc * CH_TOK + (ti + 1) * P],
                            in_=xrows[ti][:, fc * P:(fc + 1) * P],
                            transpose=True)
                    return f

                def mk_mm(m, k):
                    def f():
                        if m == 0 and k == 0:
                            xg_of[ci] = xgp.tile([P, 16 * CH_TOK], BF16,
                                                 tag="xg", name="xg")
                        if k == 0:
                            px[m] = pxp.tile([P, CH_TOK], F32, space="PSUM",
                                             tag="px", name="px")
                        T, s_ = PERM[m]
                        col = T * HID + s_ * P
                        nc.tensor.matmul(
                            px[m][:],
                            lhsT=fT_sb[:, k * G4 + col:k * G4 + col + P],
                            rhs=xT[0][:, k * CH_TOK:(k + 1) * CH_TOK],
                            start=(k == 0), stop=(k == 5))
                    return f

                def mk_evac(m):
                    def f():
                        nc.vector.tensor_scalar_add(
                            xg_of[ci][:, m * CH_TOK:(m + 1) * CH_TOK],
                            px.pop(m)[:], dcol[:, m:m + 1])
                    return f

                for ti in range(ch_tile):
                    for e in range(NE):
                        ops.append(mk_gather(ti, e))
                for ti in range(ch_tile):
                    for fc in range(6):
                        ops.append(mk_tr(ti, fc))
                dve_ops = []
                for m in range(16):
                    for k in range(6):
                        ops.append(mk_mm(m, k))
                    dve_ops.append((36 + 6 * (m + 1), mk_evac(m)))
                return ops, dve_ops

            def run_ops(ops):
                for f in ops:
                    f()

            # ---------------- P2 helpers ----------------------------------
            def emit_step(t, pending):
                """One LSTM step: xg inject + MMs, pointwise; drains P1
                slivers into the PE gap. pr is split per hid-half into two
                PSUM banks so ACT reads never collide with MM writes."""
                ci, tl = divmod(t, CH_S)
                xg3 = xg_of[ci][:].rearrange("p (m tok) -> p m tok", m=16)
                # full-bank (512 f32) tiles so two pr buffers never share
                # a PSUM bank; only the first 64 cols are used
                pr_a = prpa.tile([P, 512], F32, space="PSUM", tag="pra",
                                 name="pra")
                pr_b = prpb.tile([P, 512], F32, space="PSUM", tag="prb",
                                 name="prb")
                halves = (pr_a, pr_b)
                for oh in (0, 1):
                    nc.tensor.matmul(
                        halves[oh][:, 0:8 * B_LOC].rearrange(
                            "p (m b) -> p m b", m=8),
                        lhsT=ident_bf[:],
                        rhs=xg3[:, oh * 8:(oh + 1) * 8,
                                tl * B_LOC:(tl + 1) * B_LOC],
                        start=True, stop=(t == 0), skip_group_check=True)
                if t > 0:
                    for oh in (0, 1):
                        pr = halves[oh]
                        for kp in (0, 1):
                            for jj in range(8):
                                j = oh * 8 + jj
                                T, s_ = PERM[j]
                                col = T * HID + s_ * P
                                for kt in (2 * kp, 2 * kp + 1):
                                    nc.tensor.matmul(
                                        pr[:, jj * B_LOC:(jj + 1) * B_LOC],
                                        lhsT=whh_sb[:, kt * G4 + col:
                                                    kt * G4 + col + P],
                                        rhs=hT[:, kt * n_tok + (t - 1) * B_LOC:
                                               kt * n_tok + t * B_LOC],
                                        start=False, stop=(kp == 1 and kt == 3),
                                        skip_group_check=True)
                # pointwise: per half, gate cols are [g g | i i f f o o].
                # ACT queue order: tg0 sig0 tc0 tg1 sig1 tc1; DVE order:
                # fc0 ig0 c0 h0 fc1 ig1 c1 h1 — h0 as early as possible so
                # the next step's kt01 matmuls start while half 1 finishes.
                HB = B_LOC  # 8
                sifo = [None, None]
                tg = [None, None]
                tc2 = [None, None]

                def act_half(oh):
                    prh = halves[oh]
                    sifo[oh] = pwp.tile([P, 6 * HB], F32, tag=f"sifo{oh}", name="sifo")
                    nc.scalar.activation(sifo[oh][:], prh[:, 0:6 * HB],
                                         AF.Sigmoid, scale=inv_s)
                    tg[oh] = pwp.tile([P, 2 * HB], F32, tag=f"tg{oh}", name="tg")
                    nc.scalar.activation(tg[oh][:], prh[:, 6 * HB:8 * HB],
                                         AF.Tanh, scale=inv_s)

                def dve_half(oh):
                    si = sifo[oh][:, 0:2 * HB]
                    sf = sifo[oh][:, 2 * HB:4 * HB]
                    cs = c_sb[:, oh * 2 * HB:(oh + 1) * 2 * HB]
                    fc = pwp.tile([P, 2 * HB], F32, tag=f"fc{oh}", name="fc")
                    nc.vector.tensor_tensor(out=fc[:], in0=sf, in1=cs,
                                            op=ALU.mult)
                    ig = pwp.tile([P, 2 * HB], F32, tag=f"ig{oh}", name="ig")
                    nc.vector.tensor_tensor(out=ig[:], in0=si, in1=tg[oh][:],
                                            op=ALU.mult)
                    nc.vector.tensor_add(out=cs, in0=fc[:], in1=ig[:])

                def act_tc(oh):
                    cs = c_sb[:, oh * 2 * HB:(oh + 1) * 2 * HB]
                    tc2[oh] = pwp.tile([P, 2 * HB], F32, tag=f"tc{oh}", name="tc2")
                    nc.scalar.activation(tc2[oh][:], cs, AF.Tanh)

                def dve_h(oh):
                    so = sifo[oh][:, 4 * HB:6 * HB]
                    nc.vector.tensor_tensor(
                        out=hT[:].rearrange("p (kt n) -> p kt n", kt=4)
                            [:, 2 * oh:2 * oh + 2,
                             t * B_LOC:(t + 1) * B_LOC],
                        in0=so.rearrange("p (kt b) -> p kt b", kt=2),
                        in1=tc2[oh][:].rearrange("p (kt b) -> p kt b", kt=2),
                        op=ALU.mult)

                act_half(0)
                dve_half(0)
                act_tc(0)
                dve_h(0)
                act_half(1)
                dve_half(1)
                act_tc(1)
                dve_h(1)

                # P1 slivers for the next chunk: PE/DMA ops are emitted after
                # the pointwise so they fill the PE gap without inserting
                # into the DVE dependency chain; evacs drain once their
                # producing matmuls are emitted.
                n = min(len(pending[0]), 3)
                for _ in range(n):
                    pending[0].pop(0)()
                pending[2] += n
                while pending[1] and pending[1][0][0] <= pending[2]:
                    pending[1].pop(0)[1]()

            # ---------------- main schedule -------------------------------
            pe0, dve0 = p1_ops(0)
            run_ops(pe0)
            run_ops([f for _, f in dve0])
            for ci in range(n_ch):
                if interleave and ci + 1 < n_ch:
                    pe, dve = p1_ops(ci + 1)
                    pending = [pe, list(dve), 0]
                else:
                    pending = [[], [], 0]
                for tl in range(CH_S):
                    emit_step(ci * CH_S + tl, pending)
                run_ops(pending[0])
                run_ops([f for _, f in pending[1]])
                if not interleave and ci + 1 < n_ch:
                    pe, dve = p1_ops(ci + 1)
                    run_ops(pe)
                    run_ops([f for _, f in dve])

            pctx.close()

            # ---------------- P3: output projection -----------------------
            with tc.tile_pool(name="p3", bufs=2) as p3, \
                 tc.tile_pool(name="psum_o", bufs=2, space="PSUM") as psum_o:
                oT = p3.tile([TAGP2, n_tok], F32, tag="oT")
                CH_O = 512
                for ci in range(n_tok // CH_O):
                    po = psum_o.tile([TAGP2, CH_O], F32, space="PSUM",
                                     tag="po")
                    for kt in range(4):
                        nc.tensor.matmul(
                            po[:],
                            lhsT=wout_sb[:, kt * TAGP2:(kt + 1) * TAGP2],
                            rhs=hT[:, kt * n_tok + ci * CH_O:
                                   kt * n_tok + (ci + 1) * CH_O],
                            start=(kt == 0), stop=(kt == 3))
                    nc.vector.tensor_scalar_add(
                        oT[:, ci * CH_O:(ci + 1) * CH_O], po[:],
                        bout_sb[:, 0:1])
                nc.sync.dma_start(out=outp.ap(), in_=oT[:])

    nc.compile()
    return nc


# --------------------------------------------------------------------------
_NC_CACHE = {}


def _get_nc(S, V, whh_fp8=True, interleave=True, num_devices=N_CORES):
    key = (S, V, whh_fp8, interleave, num_devices)
    if key not in _NC_CACHE:
        _NC_CACHE[key] = build_nc(S, V, whh_fp8, interleave, num_devices)
    return _NC_CACHE[key]


def _ktile(a, nk, f):
    # [nk*128, f] -> [128, nk*f] with k tiles side by side
    return np.ascontiguousarray(
        a.reshape(nk, P, f).transpose(1, 0, 2).reshape(P, nk * f))


def _prep_core_inputs(c, token_ids, tables_flat, arch_params, w1, b1,
                      wih_f, whh_f, bih_f, bhh_f, wih_r, whh_r, bih_r, bhh_r,
                      wout, bout, S, V, whh_fp8):
    d, g = divmod(c, 4)
    ids = token_ids[g * B_LOC:(g + 1) * B_LOC, :]
    if d == 1:
        ids = ids[:, ::-1]
    flat = ids.T.reshape(-1).astype(np.int64)      # s-major [S*B]
    n_tile = flat.shape[0] // P
    base = flat.reshape(n_tile, P)
    gidx = (base[:, :, None] + (np.arange(NE) * V)[None, None, :])
    gidx = gidx.transpose(1, 0, 2).reshape(P, n_tile * NE).astype(np.int32)

    wih = wih_f if d == 0 else wih_r
    whh = whh_f if d == 0 else whh_r
    bih = bih_f if d == 0 else bih_r
    bhh = bhh_f if d == 0 else bhh_r

    s = WHH_SCALE if whh_fp8 else 1.0
    # softmax(arch) folded into W1 rows; W1 fused into Wih; x64 scale
    w = np.exp(arch_params - arch_params.max())
    w = (w / w.sum()).astype(np.float32)
    w1s = w1 * np.repeat(w, DE)[:, None]           # [768, 512]
    F = (w1s @ wih.T) * s                          # [768, 2048]
    dvec = (bih + bhh + wih @ b1) * s              # [2048]
    dg = np.zeros((P, 16), np.float32)
    for m, (T, s_) in enumerate(PERM):
        dg[:, m] = dvec[T * HID + s_ * P: T * HID + (s_ + 1) * P]

    whhT = np.ascontiguousarray(whh.T)             # [512, 2048]
    wdt = ml_dtypes.float8_e4m3 if whh_fp8 else ml_dtypes.bfloat16

    return {
        "tables": tables_flat,
        "gidx": gidx,
        "fT": _ktile(F, 6, G4).astype(ml_dtypes.bfloat16),
        "whhT": _ktile(whhT * s, 4, G4).astype(wdt),
        "wout": _ktile(wout[d * HID:(d + 1) * HID, :], 4, TAGP2).astype(
            ml_dtypes.bfloat16),
        "dg": dg,
        "boutc": (bout.reshape(TAGP2, 1).astype(np.float32) if d == 0
                  else np.zeros((TAGP2, 1), np.float32)),
    }


def run_cores(token_ids, emb_tables, arch_params, W1, b1,
              Wih_f, Whh_f, bih_f, bhh_f, Wih_r, Whh_r, bih_r, bhh_r,
              Wout, bout, *, whh_fp8=True, interleave=True, trace=False):
    global LAST_EXEC_NS
    B, S = token_ids.shape
    V = emb_tables.shape[1]
    assert B == 32 and emb_tables.shape[0] == NE and emb_tables.shape[2] == DE

    import time as _time
    _t0 = _time.time()
    nc = _get_nc(S, V, whh_fp8, interleave)
    _t1 = _time.time()
    tables_flat = np.ascontiguousarray(
        np.asarray(emb_tables, dtype=np.float32).reshape(NE * V, DE))

    args = (np.asarray(token_ids), tables_flat,
            np.asarray(arch_params, dtype=np.float32),
            np.asarray(W1, dtype=np.float32), np.asarray(b1, dtype=np.float32),
            np.asarray(Wih_f, dtype=np.float32),
            np.asarray(Whh_f, dtype=np.float32),
            np.asarray(bih_f, dtype=np.float32),
            np.asarray(bhh_f, dtype=np.float32),
            np.asarray(Wih_r, dtype=np.float32),
            np.asarray(Whh_r, dtype=np.float32),
            np.asarray(bih_r, dtype=np.float32),
            np.asarray(bhh_r, dtype=np.float32),
            np.asarray(Wout, dtype=np.float32),
            np.asarray(bout, dtype=np.float32))
    in_maps = [
        _prep_core_inputs(c, *args, S, V, whh_fp8) for c in range(N_CORES)
    ]
    _t2 = _time.time()
    res = run_bass_kernel_spmd(nc, in_maps, list(range(N_CORES)), trace=trace)
    LAST_EXEC_NS = res.exec_time_ns
    if os.environ.get("KERNEL_VERBOSE", "0") == "1":
        print(f"[kernel] build {_t1-_t0:.1f}s prep {_t2-_t1:.1f}s "
              f"run {_time.time()-_t2:.1f}s exec_ns={LAST_EXEC_NS}",
              flush=True)

    out = np.zeros((B, S, TAGP2), dtype=np.float32)
    for c in range(N_CORES):
        d, g = divmod(c, 4)
        part = res.results[c]["outp"]                      # [22, S*B_LOC]
        part = np.asarray(part).T.reshape(S, B_LOC, TAGP2)
        if d == 1:
            part = part[::-1]
        out[g * B_LOC:(g + 1) * B_LOC] += part.transpose(1, 0, 2)
    return out


def kernel(token_ids, emb_tables, arch_params, W1, b1,
           Wih_f, Whh_f, bih_f, bhh_f,
           Wih_r, Whh_r, bih_r, bhh_r,
           Wout, bout):
    return run_cores(
        token_ids, emb_tables, arch_params, W1, b1,
        Wih_f, Whh_f, bih_f, bhh_f, Wih_r, Whh_r, bih_r, bhh_r, Wout, bout,
        whh_fp8=os.environ.get("KERNEL_WHH_FP8", "0") == "1",
        interleave=os.environ.get("KERNEL_INTERLEAVE", "1") == "1",
        trace=os.environ.get("KERNEL_TRACE", "0") == "1",
    )
